# revision 40
# baseline (speedup 1.0000x reference)
"""RNN-T joint network kernel for 8 Trainium2 NeuronCores.

out[b,t,u,:] = W2 @ tanh(W1e @ enc[b,t] + W1d @ dec[b,u] + b1) + b2

Shapes: B=4, T=200, U=100, D=512, H=1024, O=512 (fp32 in/out).
Sharding: T split 8 ways (25 t's per core); dec + weights replicated.

All device compute is bf16 (inputs cast on host; ~5e-4 rel err, well
under the 2e-2 gate). bf16 matmuls stream at the same 1 cycle/row as
fp32r but halve LDWEIGHTS via fast-weight-load, halve the input DMA
bytes, and remove the on-device fp32->fp32r cast pass entirely.

Per-core device program:
  Phase 1: W1 split into 4 per-dk weight tiles per side so the first
           matmul can issue as soon as the first 256KB of weights lands.
           enc matmuls run dk-outer into 8 packed psum banks (enc in
           cols 0:100, dec in cols 100:500 of the same bank); dec runs
           hk-outer so the psum->sbuf copies (+b1 on the enc half)
           pipeline behind the remaining dec matmul groups.
  Phase 2: per chunk (b, up to 5 t's): two fused broadcast-add builds
           (4 h-chunks each, [p, k, t, u] APs) on DVE -> bf16 s tile,
           one tanh over [128, 8*rows] on ACT, then 4x8 accumulating
           bf16 matmuls -> psum out^T chunks, bias-add copies split
           ACT/DVE, output DMAs split across both HWDGE rings.
"""

from contextlib import ExitStack

import numpy as np
import ml_dtypes

import concourse.bacc as bacc
import concourse.bass as bass
import concourse.mybir as mybir
import concourse.tile as tile
from concourse.bass_utils import run_bass_kernel_spmd

F32 = mybir.dt.float32
BF16 = mybir.dt.bfloat16
BF16NP = ml_dtypes.bfloat16

B, T, U, D, H, O = 4, 200, 100, 512, 1024, 512
NCORES = 8
TLOC = T // NCORES            # 25 t's per core
PAIRS = B * TLOC              # 100 (b,t) pairs per core
TCH = 5                       # t's per inner chunk
CHROWS = TCH * U              # 500 rows per chunk
NCH = TLOC // TCH             # 5 chunks per b
ROWS = PAIRS * U              # 10000 output rows per core
DK = D // 128                 # 4 contraction chunks for phase 1
HK = H // 128                 # 8 h chunks
BU = B * U                    # 400

_CACHE = {}


def _build():
    nc = bacc.Bacc("TRN2", target_bir_lowering=False, debug=False,
                   num_devices=NCORES)
    # inputs arrive pre-interleaved in SBUF layout: [128, nchunk*width],
    # partition p holding chunk k's row (k*128+p) at cols [k*width, ...)
    # encT+decT ride in one buffer (one DMA: per-transfer completion
    # semaphores lag the wire by ~4us and inter-transfer bubbles cost
    # ~1.5us, so fewer transfers = earlier availability); same for biases.
    edT = nc.dram_tensor("edT", [128, DK * PAIRS + DK * BU], BF16,
                         kind="ExternalInput")
    w1eT = nc.dram_tensor("w1eT", [128, DK * H], BF16, kind="ExternalInput")
    w1dT = nc.dram_tensor("w1dT", [128, DK * H], BF16, kind="ExternalInput")
    w2T = nc.dram_tensor("w2T", [128, HK * O], BF16, kind="ExternalInput")
    biasc = nc.dram_tensor("biasc", [128, HK + O // 128], F32,
                           kind="ExternalInput")
    out = nc.dram_tensor("out", [O, ROWS], F32, kind="ExternalOutput")

    with tile.TileContext(nc) as tc, ExitStack() as ctx:
        consts = ctx.enter_context(tc.tile_pool(name="consts", bufs=1))
        spool = ctx.enter_context(tc.tile_pool(name="spool", bufs=12))
        opool = ctx.enter_context(tc.tile_pool(name="opool", bufs=8))
        psB = ctx.enter_context(tc.tile_pool(name="psB", bufs=8, space="PSUM"))

        encT_s = consts.tile([128, DK * PAIRS], BF16)
        decT_s = consts.tile([128, DK * BU], BF16)
        DOFF = DK * PAIRS                     # decT column offset in edT
        # W1 halves hk-major (host re-layout), one 512KB tile per half:
        # large transfers keep the DMA queues at full rate (~160GB/s; 128KB
        # transfers measured 2.5x slower), and the half is exactly the
        # granularity phase 2 consumes.
        # w1e's A half is further split in two so the first enc groups can
        # start ~2us earlier (right as the warm-up dummies run out).
        w1e_q = [consts.tile([128, 2 * DK * 128], BF16, name=f"w1eq{q}")
                 for q in range(2)]
        w1e_B = consts.tile([128, 4 * DK * 128], BF16)
        w1d_h = [consts.tile([128, 4 * DK * 128], BF16, name=f"w1d{h}")
                 for h in range(2)]
        w2_s = consts.tile([128, HK * O], BF16)
        bias_s = consts.tile([128, HK + O // 128], F32)
        ench_A = consts.tile([128, 4 * PAIRS], BF16)
        ench_B = consts.tile([128, 4 * PAIRS], BF16)
        dech_A = consts.tile([128, 4 * BU], BF16)
        dech_B = consts.tile([128, 4 * BU], BF16)
        scr = consts.tile([128, 512], BF16)

        # spread the 3.5MB input set across the three DMA-capable queues
        # (sync, scalar, gpsimd/SWDGE): first transfer on each queue = the
        # piece phase 1 consumes first; W2 (needed last) rides two tails.
        HD = 4 * DK * 128
        HO = 4 * O  # w2 half width
        nc.sync.dma_start(w1e_q[0][:], w1eT[:, 0:HD // 2])
        nc.sync.dma_start(w1e_q[1][:], w1eT[:, HD // 2:HD])
        nc.sync.dma_start(w1e_B[:], w1eT[:, HD:2 * HD])
        nc.sync.dma_start(w2_s[:, HO:2 * HO], w2T[:, HO:2 * HO])
        nc.scalar.dma_start(encT_s[:], edT[:, 0:DOFF])
        nc.scalar.dma_start(decT_s[:], edT[:, DOFF:])
        nc.scalar.dma_start(w1d_h[1][:], w1dT[:, HD:2 * HD])
        nc.gpsimd.dma_start(bias_s[:], biasc[:])
        nc.gpsimd.dma_start(w1d_h[0][:], w1dT[:, 0:HD])
        nc.gpsimd.dma_start(w2_s[:, 0:HO], w2T[:, 0:HO])

        # ---- PE warm-up ----
        # the tensor engine's HAM clock gate needs ~3.4us of sustained
        # activity to lift the 1.2GHz cold throttle, and re-throttles after
        # a ~3.4us idle window; burn dummy matmuls on a memset tile through
        # the whole DMA lead-in so phase 1 runs at 2.4GHz.
        nc.vector.memset(scr[:], 0.0)
        ps_w = psB.tile([128, 512], F32, tag="psB", name="ps_w")
        for _ in range(8):
            nc.tensor.matmul(ps_w[:], lhsT=scr[:, 0:128], rhs=scr[:],
                             start=True, stop=True)
        for _ in range(30):
            nc.tensor.matmul(ps_w[:, 0:128], lhsT=scr[:, 0:128],
                             rhs=scr[:, 0:128], start=True, stop=True)

        # ---- phase 1 ----
        # 8 psum banks, each packing enc (cols 0:100) + dec (cols 100:500)
        # for one h-chunk; psum->sbuf copies chase each group.
        ph = [psB.tile([128, 512], F32, tag="psB", name=f"ph{hk}")
              for hk in range(HK)]

        def enc_group(hk):
            if hk < 4:
                tile_, loc = w1e_q[hk // 2], hk % 2
            else:
                tile_, loc = w1e_B, hk % 4
            for dk in range(DK):
                nc.tensor.matmul(
                    ph[hk][:, 0:PAIRS],
                    lhsT=tile_[:, loc * 512 + dk * 128:
                               loc * 512 + (dk + 1) * 128],
                    rhs=encT_s[:, dk * PAIRS:(dk + 1) * PAIRS],
                    start=(dk == 0), stop=(dk == DK - 1),
                )
            dst = ench_A if hk < 4 else ench_B
            nc.vector.tensor_scalar_add(
                dst[:, (hk % 4) * PAIRS:(hk % 4 + 1) * PAIRS],
                ph[hk][:, 0:PAIRS], bias_s[:, hk:hk + 1])

        def dec_group(hk):
            for dk in range(DK):
                nc.tensor.matmul(
                    ph[hk][:, PAIRS:PAIRS + BU],
                    lhsT=w1d_h[hk // 4][:, (hk % 4) * 512 + dk * 128:
                                        (hk % 4) * 512 + (dk + 1) * 128],
                    rhs=decT_s[:, dk * BU:(dk + 1) * BU],
                    start=(dk == 0), stop=(dk == DK - 1),
                )
            dst = dech_A if hk < 4 else dech_B
            nc.vector.tensor_copy(
                dst[:, (hk % 4) * BU:(hk % 4 + 1) * BU],
                ph[hk][:, PAIRS:PAIRS + BU])

        # ---- phase 2 setup ----
        ench_vA = ench_A[:].rearrange("p (k t a) -> p k t a", k=4, a=1)
        ench_vB = ench_B[:].rearrange("p (k t a) -> p k t a", k=4, a=1)
        dech_vA = dech_A[:].rearrange("p (k a u) -> p k a u", k=4, a=1)
        dech_vB = dech_B[:].rearrange("p (k a u) -> p k a u", k=4, a=1)
        chunks = []
        for b in range(B):
            if b == 0:
                sizes = [1, 2, 3, 4, 5, 5, 5]
            elif b == B - 1:
                sizes = [5, 5, 5, 5, 4, 1]
            else:
                sizes = [TCH] * NCH
            t0c = 0
            for tch in sizes:
                chunks.append((b, t0c, tch))
                t0c += tch

        # s is split into per-half tiles so the first 4 matmuls of a group
        # (reading s_A) don't wait for the B-half's build+tanh.
        s_tiles = {}

        def emit_half(ci, half):
            b, t0c, tch = chunks[ci]
            rows_c = tch * U
            if ci not in s_tiles:
                s_tiles[ci] = (
                    spool.tile([128, 4 * CHROWS], BF16, tag="s",
                               name=f"sA{ci}"),
                    spool.tile([128, 4 * CHROWS], BF16, tag="s",
                               name=f"sB{ci}"),
                )
            s_t = s_tiles[ci][half]
            ench_v = ench_vA if half == 0 else ench_vB
            dech_v = dech_vA if half == 0 else dech_vB
            sv = s_t[:].rearrange("p (k t u) -> p k t u", k=4, t=TCH)
            in0 = dech_v[:, :, :, b * U:(b + 1) * U]            # [p,4,1,100]
            c0 = b * TLOC + t0c
            in1 = ench_v[:, :, c0:c0 + tch, :]                  # [p,4,tch,1]
            bc0, bc1 = bass.broadcast_tensor_aps(in0, in1)
            nc.vector.tensor_tensor(sv[:, :, 0:tch, :], bc0, bc1,
                                    mybir.AluOpType.add)
            s_half = s_t[:].rearrange("p (k c) -> p k c", k=4)[:, :, :rows_c]
            nc.scalar.activation(s_half, s_half,
                                 mybir.ActivationFunctionType.Tanh)

        # ---- phase 1 emission, in DMA arrival order ----
        for hk in range(4):
            enc_group(hk)
        for hk in range(4):
            dec_group(hk)
        # dech_A/ench_A complete here: prefetch the first A-half builds
        # while the PE chews through the B half of phase 1.
        emit_half(0, 0)
        emit_half(1, 0)
        for hk in range(4, HK):
            enc_group(hk)
        for hk in range(4, HK):
            dec_group(hk)
        emit_half(0, 1)

        # ---- phase 2 ----
        for ci, (b, t0c, tch) in enumerate(chunks):
            rows_c = tch * U
            if ci + 1 < len(chunks):
                emit_half(ci + 1, 1)
            if ci + 2 < len(chunks):
                emit_half(ci + 2, 0)
            s_A, s_B = s_tiles.pop(ci)
            row0 = b * (TLOC * U) + t0c * U
            # swapped matmul: W2 blocks stationary, s moving -> psum holds
            # out^T [o-chunk, rows]; b2 folds into the psum->sbuf copy.
            for oc in range(O // 128):
                ps = psB.tile([128, 512], F32, tag="psB")
                for k in range(HK):
                    s_t = s_A if k < 4 else s_B
                    nc.tensor.matmul(
                        ps[:, :rows_c],
                        lhsT=w2_s[:, k * O + oc * 128: k * O + (oc + 1) * 128],
                        rhs=s_t[:, (k % 4) * CHROWS: (k % 4) * CHROWS + rows_c],
                        start=(k == 0), stop=(k == HK - 1),
                    )
                ot = opool.tile([128, CHROWS], F32, tag="ot")
                if oc < 2:
                    nc.scalar.activation(
                        ot[:, :rows_c], ps[:, :rows_c],
                        mybir.ActivationFunctionType.Identity,
                        bias=bias_s[:, HK + oc:HK + oc + 1])
                else:
                    nc.vector.tensor_scalar_add(
                        ot[:, :rows_c], ps[:, :rows_c],
                        bias_s[:, HK + oc:HK + oc + 1])
                ring = nc.sync if oc % 2 == 0 else nc.scalar
                ring.dma_start(
                    out[oc * 128:(oc + 1) * 128, row0:row0 + rows_c],
                    ot[:, :rows_c])
    nc.compile()
    return nc


def kernel(enc_state, dec_state, W1, b1, W2, b2, _trace=False):
    enc_state = np.ascontiguousarray(enc_state, dtype=np.float32)
    dec_state = np.ascontiguousarray(dec_state, dtype=np.float32)
    W1 = np.asarray(W1, dtype=np.float32)
    b1 = np.asarray(b1, dtype=np.float32)
    W2 = np.asarray(W2, dtype=np.float32)
    b2 = np.asarray(b2, dtype=np.float32)

    if "nc" not in _CACHE:
        _CACHE["nc"] = _build()
    nc = _CACHE["nc"]

    def chunk128(a, dt=BF16NP):
        # [n*128, w] -> [128, n*w]: partition p holds row k*128+p of chunk k
        n = a.shape[0] // 128
        return np.ascontiguousarray(
            a.reshape(n, 128, a.shape[1]).transpose(1, 0, 2).reshape(128, -1)
            .astype(dt))

    def hk_major(w):
        # [128, DK*H] dk-major -> col (hk*DK + dk)*128 + c
        return np.ascontiguousarray(
            chunk128(w).reshape(128, DK, HK, 128)
            .transpose(0, 2, 1, 3).reshape(128, DK * H))                # [128, 8*512]

    decT = chunk128(dec_state.reshape(B * U, D).T)                      # [128, 4*400]
    w1eT = hk_major(W1[:, :D].T)
    w1dT = hk_major(W1[:, D:].T)
    w2T = chunk128(W2.T)                                                # [128, 8*O]
    biasc = np.ascontiguousarray(np.concatenate(
        [b1.reshape(HK, 128).T, b2.reshape(O // 128, 128).T],
        axis=1))                                                        # [128, 12]

    in_maps = []
    for c in range(NCORES):
        enc_c = enc_state[:, c * TLOC:(c + 1) * TLOC, :].reshape(PAIRS, D)
        encT_c = chunk128(enc_c.T)                                      # [128, 4*100]
        edT_c = np.ascontiguousarray(np.concatenate([encT_c, decT], axis=1))
        in_maps.append({
            "edT": edT_c, "w1eT": w1eT, "w1dT": w1dT,
            "w2T": w2T, "biasc": biasc,
        })

    res = run_bass_kernel_spmd(nc, in_maps, list(range(NCORES)), trace=_trace)
    out = np.empty((B, T, U, O), dtype=np.float32)
    for c in range(NCORES):
        # device output is transposed: [O, ROWS]
        out[:, c * TLOC:(c + 1) * TLOC] = (
            res.results[c]["out"].T.reshape(B, TLOC, U, O))
    if _trace:
        kernel.last_results = res
    return out


# revision 44
# speedup vs baseline: 1.0184x; 1.0184x over previous
"""RNN-T joint network kernel for 8 Trainium2 NeuronCores.

out[b,t,u,:] = W2 @ tanh(W1e @ enc[b,t] + W1d @ dec[b,u] + b1) + b2

Shapes: B=4, T=200, U=100, D=512, H=1024, O=512 (fp32 in/out).
Sharding: T split 8 ways (25 t's per core); dec + weights replicated.

All device compute is bf16 (inputs cast on host; ~5e-4 rel err, well
under the 2e-2 gate). bf16 matmuls stream at the same 1 cycle/row as
fp32r but halve LDWEIGHTS via fast-weight-load, halve the input DMA
bytes, and remove the on-device fp32->fp32r cast pass entirely.

Per-core device program:
  Phase 1: W1 split into 4 per-dk weight tiles per side so the first
           matmul can issue as soon as the first 256KB of weights lands.
           enc matmuls run dk-outer into 8 packed psum banks (enc in
           cols 0:100, dec in cols 100:500 of the same bank); dec runs
           hk-outer so the psum->sbuf copies (+b1 on the enc half)
           pipeline behind the remaining dec matmul groups.
  Phase 2: per chunk (b, up to 5 t's): two fused broadcast-add builds
           (4 h-chunks each, [p, k, t, u] APs) on DVE -> bf16 s tile,
           one tanh over [128, 8*rows] on ACT, then 4x8 accumulating
           bf16 matmuls -> psum out^T chunks, bias-add copies split
           ACT/DVE, output DMAs split across both HWDGE rings.
"""

from contextlib import ExitStack

import numpy as np
import ml_dtypes

import concourse.bacc as bacc
import concourse.bass as bass
import concourse.mybir as mybir
import concourse.tile as tile
from concourse.bass_utils import run_bass_kernel_spmd

F32 = mybir.dt.float32
BF16 = mybir.dt.bfloat16
BF16NP = ml_dtypes.bfloat16

B, T, U, D, H, O = 4, 200, 100, 512, 1024, 512
NCORES = 8
TLOC = T // NCORES            # 25 t's per core
PAIRS = B * TLOC              # 100 (b,t) pairs per core
TCH = 5                       # t's per inner chunk
CHROWS = TCH * U              # 500 rows per chunk
NCH = TLOC // TCH             # 5 chunks per b
ROWS = PAIRS * U              # 10000 output rows per core
DK = D // 128                 # 4 contraction chunks for phase 1
HK = H // 128                 # 8 h chunks
BU = B * U                    # 400

_CACHE = {}


def _build():
    nc = bacc.Bacc("TRN2", target_bir_lowering=False, debug=False,
                   num_devices=NCORES)
    # inputs arrive pre-interleaved in SBUF layout: [128, nchunk*width],
    # partition p holding chunk k's row (k*128+p) at cols [k*width, ...)
    # encT+decT ride in one buffer (one DMA: per-transfer completion
    # semaphores lag the wire by ~4us and inter-transfer bubbles cost
    # ~1.5us, so fewer transfers = earlier availability); same for biases.
    edT = nc.dram_tensor("edT", [128, DK * PAIRS + DK * BU], BF16,
                         kind="ExternalInput")
    w1eT = nc.dram_tensor("w1eT", [128, DK * H], BF16, kind="ExternalInput")
    w1dT = nc.dram_tensor("w1dT", [128, DK * H], BF16, kind="ExternalInput")
    w2T = nc.dram_tensor("w2T", [128, HK * O], BF16, kind="ExternalInput")
    biasc = nc.dram_tensor("biasc", [128, HK + O // 128], F32,
                           kind="ExternalInput")
    out = nc.dram_tensor("out", [O, ROWS], F32, kind="ExternalOutput")

    with tile.TileContext(nc) as tc, ExitStack() as ctx:
        consts = ctx.enter_context(tc.tile_pool(name="consts", bufs=1))
        spool = ctx.enter_context(tc.tile_pool(name="spool", bufs=12))
        opool = ctx.enter_context(tc.tile_pool(name="opool", bufs=8))
        psB = ctx.enter_context(tc.tile_pool(name="psB", bufs=8, space="PSUM"))

        encT_s = consts.tile([128, DK * PAIRS], BF16)
        decT_s = consts.tile([128, DK * BU], BF16)
        DOFF = DK * PAIRS                     # decT column offset in edT
        # W1 halves hk-major (host re-layout), one 512KB tile per half:
        # large transfers keep the DMA queues at full rate (~160GB/s; 128KB
        # transfers measured 2.5x slower), and the half is exactly the
        # granularity phase 2 consumes.
        w1e_h = [consts.tile([128, 4 * DK * 128], BF16, name=f"w1e{h}")
                 for h in range(2)]
        w1d_h = [consts.tile([128, 4 * DK * 128], BF16, name=f"w1d{h}")
                 for h in range(2)]
        w2_s = consts.tile([128, HK * O], BF16)
        bias_s = consts.tile([128, HK + O // 128], F32)
        ench_A = consts.tile([128, 4 * PAIRS], BF16)
        ench_B = consts.tile([128, 4 * PAIRS], BF16)
        dech_A = consts.tile([128, 4 * BU], BF16)
        dech_B = consts.tile([128, 4 * BU], BF16)
        scr = consts.tile([128, 512], BF16)

        # spread the 3.5MB input set across the three DMA-capable queues
        # (sync, scalar, gpsimd/SWDGE): first transfer on each queue = the
        # piece phase 1 consumes first; W2 (needed last) rides two tails.
        HD = 4 * DK * 128
        HO = 4 * O  # w2 half width
        nc.sync.dma_start(w1e_h[0][:], w1eT[:, 0:HD])
        nc.sync.dma_start(w1e_h[1][:], w1eT[:, HD:2 * HD])
        nc.sync.dma_start(w2_s[:, HO:2 * HO], w2T[:, HO:2 * HO])
        nc.scalar.dma_start(encT_s[:], edT[:, 0:DOFF])
        nc.scalar.dma_start(decT_s[:], edT[:, DOFF:])
        nc.scalar.dma_start(w1d_h[1][:], w1dT[:, HD:2 * HD])
        nc.gpsimd.dma_start(bias_s[:], biasc[:])
        nc.gpsimd.dma_start(w1d_h[0][:], w1dT[:, 0:HD])
        nc.gpsimd.dma_start(w2_s[:, 0:HO], w2T[:, 0:HO])

        # ---- PE warm-up ----
        # the tensor engine's HAM clock gate needs ~3.4us of sustained
        # activity to lift the 1.2GHz cold throttle, and re-throttles after
        # a ~3.4us idle window; burn dummy matmuls on a memset tile through
        # the whole DMA lead-in so phase 1 runs at 2.4GHz.
        nc.vector.memset(scr[:], 0.0)
        ps_w = psB.tile([128, 512], F32, tag="psB", name="ps_w")
        for _ in range(8):
            nc.tensor.matmul(ps_w[:], lhsT=scr[:, 0:128], rhs=scr[:],
                             start=True, stop=True)
        for _ in range(52):
            nc.tensor.matmul(ps_w[:, 0:128], lhsT=scr[:, 0:128],
                             rhs=scr[:, 0:128], start=True, stop=True)

        # ---- phase 1 ----
        # 8 psum banks, each packing enc (cols 0:100) + dec (cols 100:500)
        # for one h-chunk; psum->sbuf copies chase each group.
        ph = [psB.tile([128, 512], F32, tag="psB", name=f"ph{hk}")
              for hk in range(HK)]

        def enc_group(hk):
            for dk in range(DK):
                nc.tensor.matmul(
                    ph[hk][:, 0:PAIRS],
                    lhsT=w1e_h[hk // 4][:, (hk % 4) * 512 + dk * 128:
                                        (hk % 4) * 512 + (dk + 1) * 128],
                    rhs=encT_s[:, dk * PAIRS:(dk + 1) * PAIRS],
                    start=(dk == 0), stop=(dk == DK - 1),
                )
            dst = ench_A if hk < 4 else ench_B
            nc.vector.tensor_scalar_add(
                dst[:, (hk % 4) * PAIRS:(hk % 4 + 1) * PAIRS],
                ph[hk][:, 0:PAIRS], bias_s[:, hk:hk + 1])

        def dec_group(hk):
            for dk in range(DK):
                nc.tensor.matmul(
                    ph[hk][:, PAIRS:PAIRS + BU],
                    lhsT=w1d_h[hk // 4][:, (hk % 4) * 512 + dk * 128:
                                        (hk % 4) * 512 + (dk + 1) * 128],
                    rhs=decT_s[:, dk * BU:(dk + 1) * BU],
                    start=(dk == 0), stop=(dk == DK - 1),
                )
            dst = dech_A if hk < 4 else dech_B
            nc.vector.tensor_copy(
                dst[:, (hk % 4) * BU:(hk % 4 + 1) * BU],
                ph[hk][:, PAIRS:PAIRS + BU])

        # ---- phase 2 setup ----
        ench_vA = ench_A[:].rearrange("p (k t a) -> p k t a", k=4, a=1)
        ench_vB = ench_B[:].rearrange("p (k t a) -> p k t a", k=4, a=1)
        dech_vA = dech_A[:].rearrange("p (k a u) -> p k a u", k=4, a=1)
        dech_vB = dech_B[:].rearrange("p (k a u) -> p k a u", k=4, a=1)
        chunks = []
        for b in range(B):
            if b == 0:
                sizes = [1, 2, 3, 4, 5, 5, 5]
            elif b == B - 1:
                sizes = [5, 5, 5, 5, 4, 1]
            else:
                sizes = [TCH] * NCH
            t0c = 0
            for tch in sizes:
                chunks.append((b, t0c, tch))
                t0c += tch

        # s is split into per-half tiles so the first 4 matmuls of a group
        # (reading s_A) don't wait for the B-half's build+tanh.
        s_tiles = {}

        def emit_half(ci, half):
            b, t0c, tch = chunks[ci]
            rows_c = tch * U
            if ci not in s_tiles:
                s_tiles[ci] = (
                    spool.tile([128, 4 * CHROWS], BF16, tag="s",
                               name=f"sA{ci}"),
                    spool.tile([128, 4 * CHROWS], BF16, tag="s",
                               name=f"sB{ci}"),
                )
            s_t = s_tiles[ci][half]
            ench_v = ench_vA if half == 0 else ench_vB
            dech_v = dech_vA if half == 0 else dech_vB
            sv = s_t[:].rearrange("p (k t u) -> p k t u", k=4, t=TCH)
            in0 = dech_v[:, :, :, b * U:(b + 1) * U]            # [p,4,1,100]
            c0 = b * TLOC + t0c
            in1 = ench_v[:, :, c0:c0 + tch, :]                  # [p,4,tch,1]
            bc0, bc1 = bass.broadcast_tensor_aps(in0, in1)
            nc.vector.tensor_tensor(sv[:, :, 0:tch, :], bc0, bc1,
                                    mybir.AluOpType.add)
            s_half = s_t[:].rearrange("p (k c) -> p k c", k=4)[:, :, :rows_c]
            nc.scalar.activation(s_half, s_half,
                                 mybir.ActivationFunctionType.Tanh)

        # ---- phase 1 emission, in DMA arrival order ----
        for hk in range(4):
            enc_group(hk)
        for hk in range(4):
            dec_group(hk)
        # dech_A/ench_A complete here: prefetch the first A-half builds
        # while the PE chews through the B half of phase 1.
        emit_half(0, 0)
        emit_half(1, 0)
        for hk in range(4, HK):
            enc_group(hk)
        for hk in range(4, HK):
            dec_group(hk)
        emit_half(0, 1)

        # ---- phase 2 ----
        for ci, (b, t0c, tch) in enumerate(chunks):
            rows_c = tch * U
            if ci + 1 < len(chunks):
                emit_half(ci + 1, 1)
            if ci + 2 < len(chunks):
                emit_half(ci + 2, 0)
            s_A, s_B = s_tiles.pop(ci)
            row0 = b * (TLOC * U) + t0c * U
            # swapped matmul: W2 blocks stationary, s moving -> psum holds
            # out^T [o-chunk, rows]; b2 folds into the psum->sbuf copy.
            for oc in range(O // 128):
                ps = psB.tile([128, 512], F32, tag="psB")
                for k in range(HK):
                    s_t = s_A if k < 4 else s_B
                    nc.tensor.matmul(
                        ps[:, :rows_c],
                        lhsT=w2_s[:, k * O + oc * 128: k * O + (oc + 1) * 128],
                        rhs=s_t[:, (k % 4) * CHROWS: (k % 4) * CHROWS + rows_c],
                        start=(k == 0), stop=(k == HK - 1),
                    )
                ot = opool.tile([128, CHROWS], F32, tag="ot")
                if oc < 2:
                    nc.scalar.activation(
                        ot[:, :rows_c], ps[:, :rows_c],
                        mybir.ActivationFunctionType.Identity,
                        bias=bias_s[:, HK + oc:HK + oc + 1])
                else:
                    nc.vector.tensor_scalar_add(
                        ot[:, :rows_c], ps[:, :rows_c],
                        bias_s[:, HK + oc:HK + oc + 1])
                ring = nc.sync if oc % 2 == 0 else nc.scalar
                ring.dma_start(
                    out[oc * 128:(oc + 1) * 128, row0:row0 + rows_c],
                    ot[:, :rows_c])
    nc.compile()
    return nc


def kernel(enc_state, dec_state, W1, b1, W2, b2, _trace=False):
    enc_state = np.ascontiguousarray(enc_state, dtype=np.float32)
    dec_state = np.ascontiguousarray(dec_state, dtype=np.float32)
    W1 = np.asarray(W1, dtype=np.float32)
    b1 = np.asarray(b1, dtype=np.float32)
    W2 = np.asarray(W2, dtype=np.float32)
    b2 = np.asarray(b2, dtype=np.float32)

    if "nc" not in _CACHE:
        _CACHE["nc"] = _build()
    nc = _CACHE["nc"]

    def chunk128(a, dt=BF16NP):
        # [n*128, w] -> [128, n*w]: partition p holds row k*128+p of chunk k
        n = a.shape[0] // 128
        return np.ascontiguousarray(
            a.reshape(n, 128, a.shape[1]).transpose(1, 0, 2).reshape(128, -1)
            .astype(dt))

    def hk_major(w):
        # [128, DK*H] dk-major -> col (hk*DK + dk)*128 + c
        return np.ascontiguousarray(
            chunk128(w).reshape(128, DK, HK, 128)
            .transpose(0, 2, 1, 3).reshape(128, DK * H))                # [128, 8*512]

    decT = chunk128(dec_state.reshape(B * U, D).T)                      # [128, 4*400]
    w1eT = hk_major(W1[:, :D].T)
    w1dT = hk_major(W1[:, D:].T)
    w2T = chunk128(W2.T)                                                # [128, 8*O]
    biasc = np.ascontiguousarray(np.concatenate(
        [b1.reshape(HK, 128).T, b2.reshape(O // 128, 128).T],
        axis=1))                                                        # [128, 12]

    in_maps = []
    for c in range(NCORES):
        enc_c = enc_state[:, c * TLOC:(c + 1) * TLOC, :].reshape(PAIRS, D)
        encT_c = chunk128(enc_c.T)                                      # [128, 4*100]
        edT_c = np.ascontiguousarray(np.concatenate([encT_c, decT], axis=1))
        in_maps.append({
            "edT": edT_c, "w1eT": w1eT, "w1dT": w1dT,
            "w2T": w2T, "biasc": biasc,
        })

    res = run_bass_kernel_spmd(nc, in_maps, list(range(NCORES)), trace=_trace)
    out = np.empty((B, T, U, O), dtype=np.float32)
    for c in range(NCORES):
        # device output is transposed: [O, ROWS]
        out[:, c * TLOC:(c + 1) * TLOC] = (
            res.results[c]["out"].T.reshape(B, TLOC, U, O))
    if _trace:
        kernel.last_results = res
    return out


# revision 45
# speedup vs baseline: 1.0208x; 1.0023x over previous
"""RNN-T joint network kernel for 8 Trainium2 NeuronCores.

out[b,t,u,:] = W2 @ tanh(W1e @ enc[b,t] + W1d @ dec[b,u] + b1) + b2

Shapes: B=4, T=200, U=100, D=512, H=1024, O=512 (fp32 in/out).
Sharding: T split 8 ways (25 t's per core); dec + weights replicated.

All device compute is bf16 (inputs cast on host; ~5e-4 rel err, well
under the 2e-2 gate). bf16 matmuls stream at the same 1 cycle/row as
fp32r but halve LDWEIGHTS via fast-weight-load, halve the input DMA
bytes, and remove the on-device fp32->fp32r cast pass entirely.

Per-core device program:
  Phase 1: W1 split into 4 per-dk weight tiles per side so the first
           matmul can issue as soon as the first 256KB of weights lands.
           enc matmuls run dk-outer into 8 packed psum banks (enc in
           cols 0:100, dec in cols 100:500 of the same bank); dec runs
           hk-outer so the psum->sbuf copies (+b1 on the enc half)
           pipeline behind the remaining dec matmul groups.
  Phase 2: per chunk (b, up to 5 t's): two fused broadcast-add builds
           (4 h-chunks each, [p, k, t, u] APs) on DVE -> bf16 s tile,
           one tanh over [128, 8*rows] on ACT, then 4x8 accumulating
           bf16 matmuls -> psum out^T chunks, bias-add copies split
           ACT/DVE, output DMAs split across both HWDGE rings.
"""

from contextlib import ExitStack

import numpy as np
import ml_dtypes

import concourse.bacc as bacc
import concourse.bass as bass
import concourse.mybir as mybir
import concourse.tile as tile
from concourse.bass_utils import run_bass_kernel_spmd

F32 = mybir.dt.float32
BF16 = mybir.dt.bfloat16
BF16NP = ml_dtypes.bfloat16

B, T, U, D, H, O = 4, 200, 100, 512, 1024, 512
NCORES = 8
TLOC = T // NCORES            # 25 t's per core
PAIRS = B * TLOC              # 100 (b,t) pairs per core
TCH = 5                       # t's per inner chunk
CHROWS = TCH * U              # 500 rows per chunk
NCH = TLOC // TCH             # 5 chunks per b
ROWS = PAIRS * U              # 10000 output rows per core
DK = D // 128                 # 4 contraction chunks for phase 1
HK = H // 128                 # 8 h chunks
BU = B * U                    # 400

_CACHE = {}


def _build():
    nc = bacc.Bacc("TRN2", target_bir_lowering=False, debug=False,
                   num_devices=NCORES)
    # inputs arrive pre-interleaved in SBUF layout: [128, nchunk*width],
    # partition p holding chunk k's row (k*128+p) at cols [k*width, ...)
    # encT+decT ride in one buffer (one DMA: per-transfer completion
    # semaphores lag the wire by ~4us and inter-transfer bubbles cost
    # ~1.5us, so fewer transfers = earlier availability); same for biases.
    edT = nc.dram_tensor("edT", [128, DK * PAIRS + DK * BU], BF16,
                         kind="ExternalInput")
    w1eT = nc.dram_tensor("w1eT", [128, DK * H], BF16, kind="ExternalInput")
    w1dT = nc.dram_tensor("w1dT", [128, DK * H], BF16, kind="ExternalInput")
    w2T = nc.dram_tensor("w2T", [128, HK * O], BF16, kind="ExternalInput")
    biasc = nc.dram_tensor("biasc", [128, HK + O // 128], F32,
                           kind="ExternalInput")
    out = nc.dram_tensor("out", [O, ROWS], F32, kind="ExternalOutput")

    with tile.TileContext(nc) as tc, ExitStack() as ctx:
        consts = ctx.enter_context(tc.tile_pool(name="consts", bufs=1))
        spool = ctx.enter_context(tc.tile_pool(name="spool", bufs=12))
        opool = ctx.enter_context(tc.tile_pool(name="opool", bufs=8))
        psB = ctx.enter_context(tc.tile_pool(name="psB", bufs=8, space="PSUM"))

        encT_s = consts.tile([128, DK * PAIRS], BF16)
        decT_s = consts.tile([128, DK * BU], BF16)
        DOFF = DK * PAIRS                     # decT column offset in edT
        # W1 halves hk-major (host re-layout), one 512KB tile per half:
        # large transfers keep the DMA queues at full rate (~160GB/s; 128KB
        # transfers measured 2.5x slower), and the half is exactly the
        # granularity phase 2 consumes.
        w1e_h = [consts.tile([128, 4 * DK * 128], BF16, name=f"w1e{h}")
                 for h in range(2)]
        w1d_h = [consts.tile([128, 4 * DK * 128], BF16, name=f"w1d{h}")
                 for h in range(2)]
        w2_s = consts.tile([128, HK * O], BF16)
        bias_s = consts.tile([128, HK + O // 128], F32)
        ench_A = consts.tile([128, 4 * PAIRS], BF16)
        ench_B = consts.tile([128, 4 * PAIRS], BF16)
        dech_A = consts.tile([128, 4 * BU], BF16)
        dech_B = consts.tile([128, 4 * BU], BF16)
        scr = consts.tile([128, 512], BF16)

        # spread the 3.5MB input set across the three DMA-capable queues
        # (sync, scalar, gpsimd/SWDGE): first transfer on each queue = the
        # piece phase 1 consumes first; W2 (needed last) rides two tails.
        HD = 4 * DK * 128
        HO = 4 * O  # w2 half width
        nc.sync.dma_start(w1e_h[0][:], w1eT[:, 0:HD])
        nc.sync.dma_start(w1e_h[1][:], w1eT[:, HD:2 * HD])
        nc.sync.dma_start(w2_s[:, HO:2 * HO], w2T[:, HO:2 * HO])
        nc.scalar.dma_start(encT_s[:], edT[:, 0:DOFF])
        nc.scalar.dma_start(decT_s[:], edT[:, DOFF:])
        nc.scalar.dma_start(w1d_h[1][:], w1dT[:, HD:2 * HD])
        nc.gpsimd.dma_start(bias_s[:], biasc[:])
        nc.gpsimd.dma_start(w1d_h[0][:], w1dT[:, 0:HD])
        nc.gpsimd.dma_start(w2_s[:, 0:HO], w2T[:, 0:HO])

        # ---- PE warm-up ----
        # the tensor engine's HAM clock gate needs ~3.4us of sustained
        # activity to lift the 1.2GHz cold throttle, and re-throttles after
        # a ~3.4us idle window; burn dummy matmuls on a memset tile through
        # the whole DMA lead-in so phase 1 runs at 2.4GHz.
        nc.vector.memset(scr[:], 0.0)
        ps_w = psB.tile([128, 512], F32, tag="psB", name="ps_w")
        for _ in range(8):
            nc.tensor.matmul(ps_w[:], lhsT=scr[:, 0:128], rhs=scr[:],
                             start=True, stop=True)
        for _ in range(100):
            nc.tensor.matmul(ps_w[:, 0:128], lhsT=scr[:, 0:128],
                             rhs=scr[:, 0:128], start=True, stop=True)

        # ---- phase 1 ----
        # 8 psum banks, each packing enc (cols 0:100) + dec (cols 100:500)
        # for one h-chunk; psum->sbuf copies chase each group.
        ph = [psB.tile([128, 512], F32, tag="psB", name=f"ph{hk}")
              for hk in range(HK)]

        def enc_group(hk):
            for dk in range(DK):
                nc.tensor.matmul(
                    ph[hk][:, 0:PAIRS],
                    lhsT=w1e_h[hk // 4][:, (hk % 4) * 512 + dk * 128:
                                        (hk % 4) * 512 + (dk + 1) * 128],
                    rhs=encT_s[:, dk * PAIRS:(dk + 1) * PAIRS],
                    start=(dk == 0), stop=(dk == DK - 1),
                )
            dst = ench_A if hk < 4 else ench_B
            nc.vector.tensor_scalar_add(
                dst[:, (hk % 4) * PAIRS:(hk % 4 + 1) * PAIRS],
                ph[hk][:, 0:PAIRS], bias_s[:, hk:hk + 1])

        def dec_group(hk):
            for dk in range(DK):
                nc.tensor.matmul(
                    ph[hk][:, PAIRS:PAIRS + BU],
                    lhsT=w1d_h[hk // 4][:, (hk % 4) * 512 + dk * 128:
                                        (hk % 4) * 512 + (dk + 1) * 128],
                    rhs=decT_s[:, dk * BU:(dk + 1) * BU],
                    start=(dk == 0), stop=(dk == DK - 1),
                )
            dst = dech_A if hk < 4 else dech_B
            nc.vector.tensor_copy(
                dst[:, (hk % 4) * BU:(hk % 4 + 1) * BU],
                ph[hk][:, PAIRS:PAIRS + BU])

        # ---- phase 2 setup ----
        ench_vA = ench_A[:].rearrange("p (k t a) -> p k t a", k=4, a=1)
        ench_vB = ench_B[:].rearrange("p (k t a) -> p k t a", k=4, a=1)
        dech_vA = dech_A[:].rearrange("p (k a u) -> p k a u", k=4, a=1)
        dech_vB = dech_B[:].rearrange("p (k a u) -> p k a u", k=4, a=1)
        chunks = []
        for b in range(B):
            if b == 0:
                sizes = [1, 2, 3, 4, 5, 5, 5]
            elif b == B - 1:
                sizes = [5, 5, 5, 5, 4, 1]
            else:
                sizes = [TCH] * NCH
            t0c = 0
            for tch in sizes:
                chunks.append((b, t0c, tch))
                t0c += tch

        # s is split into per-half tiles so the first 4 matmuls of a group
        # (reading s_A) don't wait for the B-half's build+tanh.
        s_tiles = {}

        def emit_half(ci, half):
            b, t0c, tch = chunks[ci]
            rows_c = tch * U
            if ci not in s_tiles:
                s_tiles[ci] = (
                    spool.tile([128, 4 * CHROWS], BF16, tag="s",
                               name=f"sA{ci}"),
                    spool.tile([128, 4 * CHROWS], BF16, tag="s",
                               name=f"sB{ci}"),
                )
            s_t = s_tiles[ci][half]
            ench_v = ench_vA if half == 0 else ench_vB
            dech_v = dech_vA if half == 0 else dech_vB
            sv = s_t[:].rearrange("p (k t u) -> p k t u", k=4, t=TCH)
            in0 = dech_v[:, :, :, b * U:(b + 1) * U]            # [p,4,1,100]
            c0 = b * TLOC + t0c
            in1 = ench_v[:, :, c0:c0 + tch, :]                  # [p,4,tch,1]
            bc0, bc1 = bass.broadcast_tensor_aps(in0, in1)
            nc.vector.tensor_tensor(sv[:, :, 0:tch, :], bc0, bc1,
                                    mybir.AluOpType.add)
            s_half = s_t[:].rearrange("p (k c) -> p k c", k=4)[:, :, :rows_c]
            nc.scalar.activation(s_half, s_half,
                                 mybir.ActivationFunctionType.Tanh)

        # ---- phase 1 emission, in DMA arrival order ----
        for hk in range(4):
            enc_group(hk)
        for hk in range(4):
            dec_group(hk)
        # dech_A/ench_A complete here: prefetch the first A-half builds
        # while the PE chews through the B half of phase 1.
        emit_half(0, 0)
        emit_half(1, 0)
        for hk in range(4, HK):
            enc_group(hk)
        for hk in range(4, HK):
            dec_group(hk)
        emit_half(0, 1)

        # ---- phase 2 ----
        for ci, (b, t0c, tch) in enumerate(chunks):
            rows_c = tch * U
            if ci + 1 < len(chunks):
                emit_half(ci + 1, 1)
            if ci + 2 < len(chunks):
                emit_half(ci + 2, 0)
            s_A, s_B = s_tiles.pop(ci)
            row0 = b * (TLOC * U) + t0c * U
            # swapped matmul: W2 blocks stationary, s moving -> psum holds
            # out^T [o-chunk, rows]; b2 folds into the psum->sbuf copy.
            for oc in range(O // 128):
                ps = psB.tile([128, 512], F32, tag="psB")
                for k in range(HK):
                    s_t = s_A if k < 4 else s_B
                    nc.tensor.matmul(
                        ps[:, :rows_c],
                        lhsT=w2_s[:, k * O + oc * 128: k * O + (oc + 1) * 128],
                        rhs=s_t[:, (k % 4) * CHROWS: (k % 4) * CHROWS + rows_c],
                        start=(k == 0), stop=(k == HK - 1),
                    )
                ot = opool.tile([128, CHROWS], F32, tag="ot")
                if oc < 2:
                    nc.scalar.activation(
                        ot[:, :rows_c], ps[:, :rows_c],
                        mybir.ActivationFunctionType.Identity,
                        bias=bias_s[:, HK + oc:HK + oc + 1])
                else:
                    nc.vector.tensor_scalar_add(
                        ot[:, :rows_c], ps[:, :rows_c],
                        bias_s[:, HK + oc:HK + oc + 1])
                ring = nc.sync if oc % 2 == 0 else nc.scalar
                ring.dma_start(
                    out[oc * 128:(oc + 1) * 128, row0:row0 + rows_c],
                    ot[:, :rows_c])
    nc.compile()
    return nc


def kernel(enc_state, dec_state, W1, b1, W2, b2, _trace=False):
    enc_state = np.ascontiguousarray(enc_state, dtype=np.float32)
    dec_state = np.ascontiguousarray(dec_state, dtype=np.float32)
    W1 = np.asarray(W1, dtype=np.float32)
    b1 = np.asarray(b1, dtype=np.float32)
    W2 = np.asarray(W2, dtype=np.float32)
    b2 = np.asarray(b2, dtype=np.float32)

    if "nc" not in _CACHE:
        _CACHE["nc"] = _build()
    nc = _CACHE["nc"]

    def chunk128(a, dt=BF16NP):
        # [n*128, w] -> [128, n*w]: partition p holds row k*128+p of chunk k
        n = a.shape[0] // 128
        return np.ascontiguousarray(
            a.reshape(n, 128, a.shape[1]).transpose(1, 0, 2).reshape(128, -1)
            .astype(dt))

    def hk_major(w):
        # [128, DK*H] dk-major -> col (hk*DK + dk)*128 + c
        return np.ascontiguousarray(
            chunk128(w).reshape(128, DK, HK, 128)
            .transpose(0, 2, 1, 3).reshape(128, DK * H))                # [128, 8*512]

    decT = chunk128(dec_state.reshape(B * U, D).T)                      # [128, 4*400]
    w1eT = hk_major(W1[:, :D].T)
    w1dT = hk_major(W1[:, D:].T)
    w2T = chunk128(W2.T)                                                # [128, 8*O]
    biasc = np.ascontiguousarray(np.concatenate(
        [b1.reshape(HK, 128).T, b2.reshape(O // 128, 128).T],
        axis=1))                                                        # [128, 12]

    in_maps = []
    for c in range(NCORES):
        enc_c = enc_state[:, c * TLOC:(c + 1) * TLOC, :].reshape(PAIRS, D)
        encT_c = chunk128(enc_c.T)                                      # [128, 4*100]
        edT_c = np.ascontiguousarray(np.concatenate([encT_c, decT], axis=1))
        in_maps.append({
            "edT": edT_c, "w1eT": w1eT, "w1dT": w1dT,
            "w2T": w2T, "biasc": biasc,
        })

    res = run_bass_kernel_spmd(nc, in_maps, list(range(NCORES)), trace=_trace)
    out = np.empty((B, T, U, O), dtype=np.float32)
    for c in range(NCORES):
        # device output is transposed: [O, ROWS]
        out[:, c * TLOC:(c + 1) * TLOC] = (
            res.results[c]["out"].T.reshape(B, TLOC, U, O))
    if _trace:
        kernel.last_results = res
    return out
